# revision 15
# baseline (speedup 1.0000x reference)
"""Trainium2 Bass kernel for nn_EnhancedSeasonalModule.

Computation (reference):
  cyc[b,s,:]   = cycle_data[(cycle_index[b]+s) % CL]
  combined     = seasonal * cyc                              [B,S,N,C]
  transformed  = combined @ W_c^T + (lin_b + b_c)            (einsum bsnc,dc->bsnd)
  z            = depthwise_conv1d_k3_same(transformed, conv_w)  over s, per (b,n)
  y            = gelu_exact(z + conv_b)
  ln           = layernorm_C(y) * ln_w + ln_b
  out          = seasonal + gamma * ln

Strategy: data-parallel over batch (2 of 16 per core, 8 cores).
Per (b,n) tile [C=128 x S=288], channels on partitions:
  - PE transposes [s,c] DMA tiles into [c,s]; DVE evacuates PSUM fused with
    the cyc multiply; conv folded into 3 PSUM-accumulated matmuls with
    A_k = diag(conv_w[:,0,k]) @ W_c; ACT applies exact GELU with the linear
    bias folded in (rank-1 matmul corrections at the s edges); LN channel
    sums come from shifted-ones matmuls accumulated into per-batch PSUM
    stats banks; the finish transposes y back to token layout and applies
    out = y^T * rstd[s] + (x + q[s]) with one fused scalar_tensor_tensor.
"""

import numpy as np
from contextlib import ExitStack

import concourse.bass as bass
import concourse.bacc as bacc_mod
import concourse.tile as tile
from concourse import mybir
from concourse.bass_utils import run_bass_kernel_spmd
from concourse.masks import make_identity

F32 = mybir.dt.float32
AF = mybir.ActivationFunctionType
OP = mybir.AluOpType

B, S, N, C, CL = 16, 288, 170, 128, 24
LN_EPS = 1e-5
NCORES = 8


def _chunks(s_total):
    out = []
    s0 = 0
    while s0 < s_total:
        sc = min(128, s_total - s0)
        out.append((s0, sc))
        s0 += sc
    return out


def build_program(
    b_per_core: int,
    n_total: int,
    s_total: int,
    gamma_sc: float,
    tb_max: int = 32,
    nb: int = 16,
    use_f32r: bool = False,
    fast_path: bool = True,
    act_fn=None,
    repeat: int = 1,
):
    """Build the SPMD Bass program for one core.

    gamma_sc: gamma * ln_w[0] when fast_path (uniform ln_w, zero ln_b);
              otherwise the per-channel scale rides in vecs[3].
    """
    nc = bacc_mod.Bacc("TRN2", target_bir_lowering=False)
    if act_fn is None:
        act_fn = AF.Gelu

    CH = _chunks(s_total)

    x_d = nc.declare_dram_parameter("x", [b_per_core, s_total, n_total, C], F32, isOutput=False)
    cyct_d = nc.declare_dram_parameter("cyct", [b_per_core, C, s_total], F32, isOutput=False)
    # a3 laid out host-side as [C, 3, C] so each A_kT slice is [c(part), d(free)]
    a3_dt = mybir.dt.float32r if use_f32r else F32
    a3_d = nc.declare_dram_parameter("a3", [C, 3, C], a3_dt, isOutput=False)
    # rows: 0=e0 (left edge corr), 1=e2 (right edge corr), 2=gbias (gelu bias),
    # 3=sc_row (gamma*ln_w), 4=gb_row (gamma*ln_b), 5=gbias again (col load)
    vec_d = nc.declare_dram_parameter("vecs", [5, C], F32, isOutput=False)
    onespad_d = nc.declare_dram_parameter("onespad", [C, 2 * tb_max], a3_dt, isOutput=False)
    out_d = nc.declare_dram_parameter("out", [b_per_core, s_total, n_total, C], F32, isOutput=True)

    def n_batches():
        res = []
        n0 = 0
        while n0 < n_total:
            t = min(tb_max, n_total - n0)
            res.append((n0, t))
            n0 += t
        return res

    mmdt = mybir.dt.float32r if use_f32r else F32
    # dtype for tiles that feed f32r matmuls (producers must round to f32r)
    RDT = mmdt

    def mm_ap(ap):
        return ap.bitcast(mmdt) if use_f32r else ap

    with tile.TileContext(nc) as tc, ExitStack() as ctx:
        singles = ctx.enter_context(tc.tile_pool(name="singles", bufs=1))
        xin = ctx.enter_context(tc.tile_pool(name="xin", bufs=9))
        ypool = ctx.enter_context(tc.tile_pool(name="ypool", bufs=tb_max + 2))
        y2pool = ctx.enter_context(tc.tile_pool(name="y2pool", bufs=2))
        comb = ctx.enter_context(tc.tile_pool(name="comb", bufs=3))
        cycp = ctx.enter_context(tc.tile_pool(name="cycp", bufs=2))
        ostage = ctx.enter_context(tc.tile_pool(name="ostage", bufs=2 * len(CH)))
        stats = ctx.enter_context(tc.tile_pool(name="stats", bufs=8))
        statsT = ctx.enter_context(tc.tile_pool(name="statsT", bufs=2 * len(CH) * 2))
        xtmp = ctx.enter_context(tc.tile_pool(name="xtmp", bufs=4))

        pxT = ctx.enter_context(tc.tile_pool(name="pxT", bufs=2, space="PSUM"))
        pz = ctx.enter_context(tc.tile_pool(name="pz", bufs=2, space="PSUM"))
        pstat = ctx.enter_context(tc.tile_pool(name="pstat", bufs=2, space="PSUM"))
        ptok = ctx.enter_context(tc.tile_pool(name="ptok", bufs=2, space="PSUM"))

        # --- constants ---
        ident = singles.tile([128, 128], F32)
        make_identity(nc, ident[:, :])

        a3_sb = singles.tile([C, 3, C], a3_dt)
        nc.sync.dma_start(out=a3_sb[:, :, :], in_=a3_d[:, :, :])

        vec_sb = singles.tile([5, C], F32)
        nc.sync.dma_start(out=vec_sb[:, :], in_=vec_d[:, :])
        # matmul lhsT operands must start at partition 0: single-row copies
        e0_row = singles.tile([1, C], F32)
        nc.sync.dma_start(out=e0_row[:, :], in_=vec_d[0:1, :])
        e2_row = singles.tile([1, C], F32)
        nc.sync.dma_start(out=e2_row[:, :], in_=vec_d[1:2, :])
        sc_row_sb = singles.tile([1, C], F32)
        nc.sync.dma_start(out=sc_row_sb[:, :], in_=vec_d[3:4, :])
        # gelu bias as a per-partition column [C,1]
        gbias_col = singles.tile([C, 1], F32)
        nc.sync.dma_start(out=gbias_col[:, :], in_=vec_d[2:3, :].rearrange("a c -> c a"))

        # memset order matters: ones_pad LAST so the PE warm-up transpose that
        # reads it observes every DVE memset tick (walrus allows only ONE sync
        # wait per matmul instruction, so steady-state matmuls must find all
        # but one of their producers already observed by the PE clock).
        ones1 = singles.tile([1, 1], F32)
        nc.vector.memset(ones1[:, :], 1.0)

        ones_col = singles.tile([128, 1], F32)
        nc.vector.memset(ones_col[:, :], 1.0)

        eps_col = singles.tile([128, 1], F32)
        nc.vector.memset(eps_col[:, :], LN_EPS)

        ones_pad = singles.tile([C, 2 * tb_max], a3_dt)
        nc.sync.dma_start(out=ones_pad[:, :], in_=onespad_d[:, :])

        # --- PE warm-up: touch each PE-read constant once, one new dep per op ---
        pwarm = ptok.tile([128, C], F32, tag="ptok", name="pwarm")
        nc.tensor.matmul(out=pwarm[:, 0:128], lhsT=ident[:, :], rhs=ident[:, :],
                         is_transpose=True, start=True, stop=True)
        opw = min(128, 2 * tb_max)
        nc.tensor.matmul(out=pwarm[0:opw, 0:128], lhsT=ones_pad[:, 0:opw].bitcast(F32), rhs=ident[:, :],
                         is_transpose=True, start=True, stop=True)
        # engine warm-up touches for DMA-loaded constants (1-wait rule)
        wscr = singles.tile([128, 1], F32)
        nc.scalar.activation(out=wscr[:, :], in_=gbias_col[:, :], func=AF.Square)
        nc.tensor.matmul(out=pwarm[:, 0:1], lhsT=a3_sb[:, 1, :].bitcast(F32), rhs=ones_col[:, :],
                         start=True, stop=True)
        nc.tensor.matmul(out=pwarm[:, 0:1], lhsT=e0_row[:, :], rhs=ones1[:, :],
                         start=True, stop=True)
        nc.tensor.matmul(out=pwarm[:, 0:1], lhsT=e2_row[:, :], rhs=ones1[:, :],
                         start=True, stop=True)
        nc.tensor.matmul(out=pwarm[:, 0:1], lhsT=sc_row_sb[:, :], rhs=ones1[:, :],
                         start=True, stop=True)

        if not fast_path:
            # gamma*ln_b broadcast across partitions, materialized once
            gb_bc = singles.tile([128, C], F32)
            src = vec_d[4, :]
            bcast = bass.AP(
                tensor=src.tensor, offset=src.offset, ap=[[0, 128]] + list(src.ap)
            )
            nc.gpsimd.dma_start(out=gb_bc[:, :], in_=bcast)

        rep_ctx = tc.For_i(0, repeat, 1) if repeat > 1 else None
        if rep_ctx is not None:
            ctx.enter_context(rep_ctx)
        for b in range(b_per_core):
            cyc_sb = cycp.tile([C, s_total], F32, tag="cycp")
            nc.sync.dma_start(out=cyc_sb[:, :], in_=cyct_d[b, :, :])
            cyc_touch = cycp.tile([128, 1], F32, tag="cyct_touch")
            nc.vector.tensor_copy(out=cyc_touch[:, :], in_=cyc_sb[:, 0:1])

            for (n0, tbn) in n_batches():
                # ---------- PHASE A ----------
                s1_ps = pstat.tile([tb_max, s_total], F32, tag="pstat")
                s2_ps = pstat.tile([tb_max, s_total], F32, tag="pstat")
                x_tiles = {}
                y_tiles = {}
                for j in range(tbn):
                    nblk = j // nb
                    if j % nb == 0:
                        nbw = min(nb, tbn - nblk * nb)
                        for chi, (s0, sc) in enumerate(CH):
                            xt = xin.tile([128, nb, C], F32, tag="xin")
                            nc.sync.dma_start(
                                out=xt[0:sc, 0:nbw, :],
                                in_=x_d[b, s0 : s0 + sc, n0 + nblk * nb : n0 + nblk * nb + nbw, :],
                            )
                            x_tiles[(nblk, chi)] = xt

                    # transpose-in -> psum [c, s]
                    xT = pxT.tile([C, s_total], F32, tag="pxT")
                    for chi, (s0, sc) in enumerate(CH):
                        nc.tensor.matmul(
                            out=xT[:, s0 : s0 + sc],
                            lhsT=x_tiles[(nblk, chi)][0:sc, j % nb, :],
                            rhs=ident[0:sc, 0:sc],
                            is_transpose=True,
                            start=True,
                            stop=True,
                        )
                    # evac + cyc multiply
                    cb = comb.tile([C, s_total], RDT, tag="comb")
                    nc.vector.tensor_tensor(out=cb[:, :], in0=xT[:, :], in1=cyc_sb[:, :], op=OP.mult)

                    # conv as 3 accumulated matmuls (+ 2 rank-1 edge corrections)
                    z = pz.tile([C, s_total], F32, tag="pz")
                    nc.tensor.matmul(
                        out=z[:, :],
                        lhsT=a3_sb[:, 1, :],
                        rhs=cb[:, :],
                        start=True,
                        stop=False,
                    )
                    if use_f32r:
                        # fp32r: even moving size + 8B-aligned PSUM offset.
                        # bulk even-aligned f32r matmuls + 1-col fp32 fixups
                        nc.tensor.matmul(
                            out=z[:, 2:s_total],
                            lhsT=a3_sb[:, 0, :],
                            rhs=cb[:, 1 : s_total - 1],
                            start=False, stop=False,
                        )
                        nc.tensor.matmul(
                            out=z[:, 1:2],
                            lhsT=a3_sb[:, 0, :].bitcast(F32),
                            rhs=cb[:, 0:1].bitcast(F32),
                            start=False, stop=False,
                        )
                        nc.tensor.matmul(
                            out=z[:, 0 : s_total - 2],
                            lhsT=a3_sb[:, 2, :],
                            rhs=cb[:, 1 : s_total - 1],
                            start=False, stop=False,
                        )
                        nc.tensor.matmul(
                            out=z[:, s_total - 2 : s_total - 1],
                            lhsT=a3_sb[:, 2, :].bitcast(F32),
                            rhs=cb[:, s_total - 1 : s_total].bitcast(F32),
                            start=False, stop=False,
                        )
                    else:
                        nc.tensor.matmul(
                            out=z[:, 1:s_total],
                            lhsT=a3_sb[:, 0, :],
                            rhs=cb[:, 0 : s_total - 1],
                            start=False,
                            stop=False,
                        )
                        nc.tensor.matmul(
                            out=z[:, 0 : s_total - 1],
                            lhsT=a3_sb[:, 2, :],
                            rhs=cb[:, 1:s_total],
                            start=False,
                            stop=False,
                        )
                    nc.tensor.matmul(
                        out=z[:, 0:1],
                        lhsT=e0_row[:, :],
                        rhs=ones1[:, :],
                        start=False,
                        stop=False,
                    )
                    nc.tensor.matmul(
                        out=z[:, s_total - 1 : s_total],
                        lhsT=e2_row[:, :],
                        rhs=ones1[:, :],
                        start=False,
                        stop=True,
                    )

                    # gelu (+ folded linear/conv bias)
                    yt = ypool.tile([C, s_total], RDT, tag="ypool")
                    nc.scalar.activation(
                        out=yt[:, :], in_=z[:, :], func=act_fn, bias=gbias_col[:, :], scale=1.0
                    )
                    y_tiles[j] = yt
                    y2 = y2pool.tile([C, s_total], RDT, tag="y2pool")
                    nc.scalar.activation(out=y2[:, :], in_=yt[:, :], func=AF.Square)

                    # stats matmuls: S1[j,:] += sum_c y ; S2[j,:] += sum_c y^2
                    win = ones_pad[:, tb_max - j : 2 * tb_max - j]
                    nc.tensor.matmul(
                        out=s1_ps[:, :], lhsT=win, rhs=yt[:, :],
                        start=(j == 0), stop=(j == tbn - 1),
                    )
                    nc.tensor.matmul(
                        out=s2_ps[:, :], lhsT=win, rhs=y2[:, :],
                        start=(j == 0), stop=(j == tbn - 1),
                    )

                # ---------- batch stats math ----------
                mu = stats.tile([tb_max, s_total], F32, tag="stats")
                nc.vector.tensor_scalar_mul(out=mu[0:tbn, :], in0=s1_ps[0:tbn, :], scalar1=1.0 / C)
                var = stats.tile([tb_max, s_total], F32, tag="stats")
                nc.vector.tensor_scalar_mul(out=var[0:tbn, :], in0=s2_ps[0:tbn, :], scalar1=1.0 / C)
                msq = stats.tile([tb_max, s_total], F32, tag="stats")
                nc.vector.tensor_tensor(out=msq[0:tbn, :], in0=mu[0:tbn, :], in1=mu[0:tbn, :], op=OP.mult)
                nc.vector.tensor_tensor(out=var[0:tbn, :], in0=var[0:tbn, :], in1=msq[0:tbn, :], op=OP.subtract)
                nc.scalar.activation(
                    out=var[0:tbn, :], in_=var[0:tbn, :], func=AF.Sqrt,
                    bias=eps_col[0:tbn, :], scale=1.0,
                )
                rstd = stats.tile([tb_max, s_total], F32, tag="stats")
                nc.vector.reciprocal(out=rstd[0:tbn, :], in_=var[0:tbn, :])

                if fast_path:
                    pp = stats.tile([tb_max, s_total], F32, tag="stats")
                    nc.vector.tensor_scalar_mul(out=pp[0:tbn, :], in0=rstd[0:tbn, :], scalar1=float(gamma_sc))
                else:
                    pp = rstd
                # q = -mu * P'
                qq = stats.tile([tb_max, s_total], F32, tag="stats")
                nc.vector.scalar_tensor_tensor(
                    out=qq[0:tbn, :], in0=mu[0:tbn, :], scalar=-1.0, in1=pp[0:tbn, :],
                    op0=OP.mult, op1=OP.mult,
                )

                # transpose P', q to [s, tb] per chunk
                ppT = {}
                qqT = {}
                for chi, (s0, sc) in enumerate(CH):
                    for name, src in (("p", pp), ("q", qq)):
                        pt = ptok.tile([128, C], F32, tag="ptok")
                        nc.tensor.matmul(
                            out=pt[0:sc, 0:tbn],
                            lhsT=src[0:tbn, s0 : s0 + sc],
                            rhs=ident[0:tbn, 0:tbn],
                            is_transpose=True,
                            start=True,
                            stop=True,
                        )
                        st = statsT.tile([128, tb_max], F32, tag="statsT")
                        nc.vector.tensor_copy(out=st[0:sc, 0:tbn], in_=pt[0:sc, 0:tbn])
                        if name == "p":
                            ppT[chi] = st
                        else:
                            qqT[chi] = st

                # ---------- PHASE B ----------
                ot = {}
                for j in range(tbn):
                    nblk = j // nb
                    if j % nb == 0:
                        for chi in range(len(CH)):
                            ot[chi] = ostage.tile([128, nb, C], F32, tag="ostage", name=f"ot{chi}")
                            nc.vector.memset(ot[chi][0:1, 0:1, 0:1], 0.0)
                    for chi, (s0, sc) in enumerate(CH):
                        bank = ptok.tile([128, C], F32, tag="ptok")
                        nc.tensor.matmul(
                            out=bank[0:sc, :],
                            lhsT=y_tiles[j][:, s0 : s0 + sc].bitcast(F32),
                            rhs=ident[0:128, 0:128],
                            is_transpose=True,
                            start=True,
                            stop=fast_path,
                        )
                        if not fast_path:
                            # bank += (-mu*rstd)[s] (x) sc_row[c]   (rank-1)
                            nc.tensor.matmul(
                                out=bank[0:sc, :],
                                lhsT=qq[j : j + 1, s0 : s0 + sc],
                                rhs=sc_row_sb[:, :],
                                start=False,
                                stop=True,
                            )
                        xp = xtmp.tile([128, C], F32, tag="xtmp")
                        if fast_path:
                            nc.vector.tensor_scalar_add(
                                out=xp[0:sc, :],
                                in0=x_tiles[(nblk, chi)][0:sc, j % nb, :],
                                scalar1=qqT[chi][0:sc, j : j + 1],
                            )
                        else:
                            nc.vector.tensor_tensor(
                                out=xp[0:sc, :],
                                in0=x_tiles[(nblk, chi)][0:sc, j % nb, :],
                                in1=gb_bc[0:sc, :],
                                op=OP.add,
                            )
                        # out = bank * P'[s] + x'
                        nc.vector.scalar_tensor_tensor(
                            out=ot[chi][0:sc, j % nb, :],
                            in0=bank[0:sc, :],
                            scalar=ppT[chi][0:sc, j : j + 1],
                            in1=xp[0:sc, :],
                            op0=OP.mult,
                            op1=OP.add,
                        )
                    if (j % nb == nb - 1) or (j == tbn - 1):
                        nbw = (j % nb) + 1
                        nst = n0 + nblk * nb
                        for chi, (s0, sc) in enumerate(CH):
                            nc.sync.dma_start(
                                out=out_d[b, s0 : s0 + sc, nst : nst + nbw, :],
                                in_=ot[chi][0:sc, 0:nbw, :],
                            )
    nc.compile()
    return nc


# ------------------------- host side -------------------------

def _host_prep(inputs):
    seasonal = np.asarray(inputs["seasonal_component"], dtype=np.float32)
    cycle_index = np.asarray(inputs["cycle_index"])
    cycle_data = np.asarray(inputs["cycle_data"], dtype=np.float32)
    W_c = np.asarray(inputs["W_c"], dtype=np.float32)
    lin_b = np.asarray(inputs["lin_b"], dtype=np.float32)
    b_c = np.asarray(inputs["b_c"], dtype=np.float32)
    conv_w = np.asarray(inputs["conv_w"], dtype=np.float32)
    conv_b = np.asarray(inputs["conv_b"], dtype=np.float32)
    ln_w = np.asarray(inputs["ln_w"], dtype=np.float32)
    ln_b = np.asarray(inputs["ln_b"], dtype=np.float32)
    gamma = float(np.asarray(inputs["gamma"]))

    b_, s_, n_, c_ = seasonal.shape
    cl = cycle_data.shape[0]

    idx = (np.asarray(cycle_index)[:, None] % cl + np.arange(s_)[None, :]) % cl
    cyc = cycle_data[idx]  # [B,S,C]
    cycT = np.ascontiguousarray(cyc.transpose(0, 2, 1))  # [B,C,S]

    w3 = conv_w[:, 0, :]  # [C,3]
    lb = lin_b + b_c
    # a3[c, k, d] = W_c.T[c,d] * w3[d,k]
    a3 = np.ascontiguousarray(
        W_c.T[:, None, :] * w3.T[None, :, :]
    ).astype(np.float32)  # [C,3,C]

    e0 = -(lb * w3[:, 0])
    e2 = -(lb * w3[:, 2])
    gbias = lb * (w3[:, 0] + w3[:, 1] + w3[:, 2]) + conv_b

    fast_path = bool(np.all(ln_w == ln_w[0]) and np.all(ln_b == 0.0))
    gamma_sc = gamma * float(ln_w[0])
    sc_row = (gamma * ln_w).astype(np.float32)
    gb_row = (gamma * ln_b).astype(np.float32)

    vecs = np.stack([e0, e2, gbias, sc_row, gb_row], axis=0).astype(np.float32)
    return seasonal, cycT, a3, vecs, fast_path, gamma_sc


def _make_onespad(tb_max=32):
    op = np.zeros((C, 2 * tb_max), np.float32)
    op[:, tb_max] = 1.0
    return op


_prog_cache = {}


def kernel(**inputs) -> np.ndarray:
    seasonal, cycT, a3, vecs, fast_path, gamma_sc = _host_prep(inputs)
    b_, s_, n_, c_ = seasonal.shape
    assert c_ == C
    bpc = b_ // NCORES

    key = (bpc, n_, s_, fast_path, gamma_sc)
    if key not in _prog_cache:
        _prog_cache[key] = build_program(
            b_per_core=bpc, n_total=n_, s_total=s_,
            gamma_sc=gamma_sc, fast_path=fast_path,
        )
    nc = _prog_cache[key]

    in_maps = []
    for i in range(NCORES):
        in_maps.append(
            {
                "x": np.ascontiguousarray(seasonal[i * bpc : (i + 1) * bpc]),
                "cyct": np.ascontiguousarray(cycT[i * bpc : (i + 1) * bpc]),
                "a3": a3,
                "vecs": vecs,
                "onespad": _make_onespad(),
            }
        )
    res = run_bass_kernel_spmd(nc, in_maps, list(range(NCORES)))
    outs = [res.results[i]["out"] for i in range(NCORES)]
    return np.concatenate(outs, axis=0)


# revision 30
# speedup vs baseline: 1.8770x; 1.8770x over previous
"""Trainium2 Bass kernel for nn_EnhancedSeasonalModule.

Computation (reference):
  cyc[b,s,:]   = cycle_data[(cycle_index[b]+s) % CL]
  combined     = seasonal * cyc                              [B,S,N,C]
  transformed  = combined @ W_c^T + (lin_b + b_c)            (einsum bsnc,dc->bsnd)
  z            = depthwise_conv1d_k3_same(transformed, conv_w)  over s, per (b,n)
  y            = gelu_exact(z + conv_b)
  ln           = layernorm_C(y) * ln_w + ln_b
  out          = seasonal + gamma * ln

Strategy: data-parallel over batch (2 of 16 per core, 8 cores).
Per (b,n) tile [C=128 x S=288], channels on partitions:
  - PE transposes [s,c] DMA tiles into [c,s]; DVE evacuates PSUM fused with
    the cyc multiply into a padded buffer whose edge columns hold
    u = -W^-1(lin_b+b_c), turning the depthwise conv + bias edge effects
    into exactly 3 PSUM-accumulated matmuls with A_k = diag(conv_w[:,0,k])@W_c;
    ACT applies exact GELU with the interior linear bias folded in; LN channel
    sums come from shifted-ones matmuls accumulated into per-batch PSUM stats
    banks; the finish transposes y back to token layout and applies
    out = y^T * rstd[s] + (x + q[s]) with one fused scalar_tensor_tensor.

fp32r matmul ISA rules (probed on hw): moving size must be even, PSUM out
offset 8B-aligned; rhs offset unrestricted. Producers feeding fp32r matmuls
must write float32r-typed outputs (engine rounds on write); DMA'd operands
must be declared float32r end-to-end.

walrus allows 1 sync wait per engine instruction; Bacc.compile()'s
generate_event_semaphores splits them, but we also warm up each PE/ACT/DVE
constant right after its DMA so steady-state instructions carry few waits.
"""

import numpy as np
from contextlib import ExitStack

import concourse.bass as bass
import concourse.bacc as bacc_mod
import concourse.tile as tile
from concourse import mybir
from concourse.bass_utils import run_bass_kernel_spmd
from concourse.masks import make_identity

F32 = mybir.dt.float32
AF = mybir.ActivationFunctionType
OP = mybir.AluOpType

B, S, N, C, CL = 16, 288, 170, 128, 24
LN_EPS = 1e-5
NCORES = 8
TB_MAX = 32


def _chunks(s_total):
    out = []
    s0 = 0
    while s0 < s_total:
        sc = min(128, s_total - s0)
        out.append((s0, sc))
        s0 += sc
    return out


def build_program(
    b_per_core: int,
    n_total: int,
    s_total: int,
    gamma_sc: float,
    tb_max: int = TB_MAX,
    nb: int = 16,
    use_f32r: bool = True,
    fast_path: bool = True,
    act_fn=None,
    repeat: int = 1,
    ablate: str = "",
    pipe_mode: int = 2,
):
    nc = bacc_mod.Bacc("TRN2", target_bir_lowering=False)
    if act_fn is None:
        act_fn = AF.Gelu

    CH = _chunks(s_total)
    SP = s_total + 2  # padded combined width (u | combined | u)

    a3_dt = mybir.dt.float32r if use_f32r else F32
    RDT = a3_dt

    x_d = nc.declare_dram_parameter("x", [b_per_core, s_total, n_total, C], F32, isOutput=False)
    cyct_d = nc.declare_dram_parameter("cyct", [b_per_core, C, s_total], F32, isOutput=False)
    a3_d = nc.declare_dram_parameter("a3", [C, 3, C], a3_dt, isOutput=False)
    # rows: 0=e0, 1=e2 (edge bias corrections), 2=gbias, 3=sc_row, 4=gb_row
    vec_d = nc.declare_dram_parameter("vecs", [5, C], F32, isOutput=False)
    onespad_d = nc.declare_dram_parameter("onespad", [C, 2 * tb_max], a3_dt, isOutput=False)
    out_d = nc.declare_dram_parameter("out", [b_per_core, s_total, n_total, C], F32, isOutput=True)

    def n_batches():
        res = []
        n0 = 0
        while n0 < n_total:
            t = min(tb_max, n_total - n0)
            res.append((n0, t))
            n0 += t
        return res

    with tile.TileContext(nc) as tc, ExitStack() as ctx:
        singles = ctx.enter_context(tc.tile_pool(name="singles", bufs=1))
        xin = ctx.enter_context(tc.tile_pool(name="xin", bufs=10))
        ypool = ctx.enter_context(tc.tile_pool(name="ypool", bufs=tb_max + 6))
        y2pool = ctx.enter_context(tc.tile_pool(name="y2pool", bufs=4))
        comb = ctx.enter_context(tc.tile_pool(name="comb", bufs=3))
        cycp = ctx.enter_context(tc.tile_pool(name="cycp", bufs=2))
        ostage = ctx.enter_context(tc.tile_pool(name="ostage", bufs=2 * len(CH)))
        stats = ctx.enter_context(tc.tile_pool(name="stats", bufs=8))
        statsT = ctx.enter_context(tc.tile_pool(name="statsT", bufs=2 * len(CH) * 2 + 2))
        xtmp = ctx.enter_context(tc.tile_pool(name="xtmp", bufs=4))

        pxT = ctx.enter_context(tc.tile_pool(name="pxT", bufs=2, space="PSUM"))
        pz = ctx.enter_context(tc.tile_pool(name="pz", bufs=2, space="PSUM"))
        pstat = ctx.enter_context(tc.tile_pool(name="pstat", bufs=2, space="PSUM"))
        ptok = ctx.enter_context(tc.tile_pool(name="ptok", bufs=2, space="PSUM"))

        # --- constants ---
        ident = singles.tile([128, 128], F32)
        make_identity(nc, ident[:, :])

        a3_sb = singles.tile([C, 3, C], a3_dt)
        nc.sync.dma_start(out=a3_sb[:, :, :], in_=a3_d[:, :, :])

        vec_sb = singles.tile([5, C], F32)
        nc.sync.dma_start(out=vec_sb[:, :], in_=vec_d[:, :])
        # zero pair for the padded combined edge columns
        u2_col = singles.tile([C, 2], F32)
        nc.vector.memset(u2_col[:, :], 0.0)
        gb_e0_col = singles.tile([C, 1], F32)
        nc.sync.dma_start(out=gb_e0_col[:, :], in_=vec_d[0:1, :].rearrange("a c -> c a"))
        gb_e2_col = singles.tile([C, 1], F32)
        nc.sync.dma_start(out=gb_e2_col[:, :], in_=vec_d[1:2, :].rearrange("a c -> c a"))
        gbias_col = singles.tile([C, 1], F32)
        nc.sync.dma_start(out=gbias_col[:, :], in_=vec_d[2:3, :].rearrange("a c -> c a"))
        sc_row_sb = singles.tile([1, C], F32)
        nc.sync.dma_start(out=sc_row_sb[:, :], in_=vec_d[3:4, :])

        ones1 = singles.tile([1, 1], F32)
        nc.vector.memset(ones1[:, :], 1.0)
        ones_col = singles.tile([128, 1], F32)
        nc.vector.memset(ones_col[:, :], 1.0)
        eps_col = singles.tile([128, 1], F32)
        nc.vector.memset(eps_col[:, :], LN_EPS)

        ones_pad = singles.tile([C, 2 * tb_max], a3_dt)
        nc.sync.dma_start(out=ones_pad[:, :], in_=onespad_d[:, :])

        # --- engine warm-ups: touch DMA'd constants once, one new dep per op ---
        pwarm = ptok.tile([128, C], F32, tag="ptok", name="pwarm")
        nc.tensor.matmul(out=pwarm[:, 0:128], lhsT=ident[:, :], rhs=ident[:, :],
                         is_transpose=True, start=True, stop=True)
        opw = min(128, 2 * tb_max)
        nc.tensor.matmul(out=pwarm[0:opw, 0:128], lhsT=ones_pad[:, 0:opw].bitcast(F32),
                         rhs=ident[:, :], is_transpose=True, start=True, stop=True)
        nc.tensor.matmul(out=pwarm[:, 0:1], lhsT=a3_sb[:, 1, :].bitcast(F32),
                         rhs=ones_col[:, :], start=True, stop=True)
        nc.tensor.matmul(out=pwarm[:, 0:1], lhsT=sc_row_sb[:, :], rhs=ones1[:, :],
                         start=True, stop=True)
        wscr = singles.tile([128, 1], F32)
        nc.scalar.activation(out=wscr[:, :], in_=gbias_col[:, :], func=AF.Square)
        nc.scalar.activation(out=wscr[:, :], in_=gb_e0_col[:, :], func=AF.Square)
        nc.scalar.activation(out=wscr[:, :], in_=gb_e2_col[:, :], func=AF.Square)
        wscr2 = singles.tile([128, 2], F32)
        nc.vector.tensor_copy(out=wscr2[:, :], in_=u2_col[:, :])

        if not fast_path:
            gb_bc = singles.tile([128, C], F32)
            src = vec_d[4, :]
            bcast = bass.AP(tensor=src.tensor, offset=src.offset, ap=[[0, 128]] + list(src.ap))
            nc.gpsimd.dma_start(out=gb_bc[:, :], in_=bcast)

        rep_ctx = tc.For_i(0, repeat, 1) if repeat > 1 else None
        if rep_ctx is not None:
            ctx.enter_context(rep_ctx)

        pipeline = (pipe_mode > 0) and not ablate

        def emit_stats_math_and_pq(st):
            tbn = st["tbn"]
            s1_ps, s2_ps = st["s1"], st["s2"]
            mu = stats.tile([tb_max, s_total], F32, tag="stats", name="mu")
            nc.vector.tensor_scalar_mul(out=mu[0:tbn, :], in0=s1_ps[0:tbn, :], scalar1=1.0 / C)
            var = stats.tile([tb_max, s_total], F32, tag="stats", name="var")
            nc.vector.tensor_scalar_mul(out=var[0:tbn, :], in0=s2_ps[0:tbn, :], scalar1=1.0 / C)
            msq = stats.tile([tb_max, s_total], F32, tag="stats", name="msq")
            nc.vector.tensor_tensor(out=msq[0:tbn, :], in0=mu[0:tbn, :], in1=mu[0:tbn, :], op=OP.mult)
            nc.vector.tensor_tensor(out=var[0:tbn, :], in0=var[0:tbn, :], in1=msq[0:tbn, :], op=OP.subtract)
            nc.scalar.activation(
                out=var[0:tbn, :], in_=var[0:tbn, :], func=AF.Sqrt,
                bias=eps_col[0:tbn, :], scale=1.0,
            )
            rstd = stats.tile([tb_max, s_total], F32, tag="stats", name="rstd")
            nc.vector.reciprocal(out=rstd[0:tbn, :], in_=var[0:tbn, :])
            if fast_path:
                pp = stats.tile([tb_max, s_total], F32, tag="stats", name="pp")
                nc.vector.tensor_scalar_mul(out=pp[0:tbn, :], in0=rstd[0:tbn, :], scalar1=float(gamma_sc))
            else:
                pp = rstd
            qq = stats.tile([tb_max, s_total], F32, tag="stats", name="qq")
            nc.vector.scalar_tensor_tensor(
                out=qq[0:tbn, :], in0=mu[0:tbn, :], scalar=-1.0, in1=pp[0:tbn, :],
                op0=OP.mult, op1=OP.mult,
            )
            st["qq"] = qq
            ppT = {}
            qqT = {}
            for chi, (s0, sc) in enumerate(CH):
                for name, srcm in (("p", pp), ("q", qq)):
                    pt = ptok.tile([128, C], F32, tag="ptok", name="pt")
                    nc.tensor.matmul(
                        out=pt[0:sc, 0:tbn],
                        lhsT=srcm[0:tbn, s0 : s0 + sc],
                        rhs=ident[0:tbn, 0:tbn],
                        is_transpose=True,
                        start=True,
                        stop=True,
                    )
                    st_t = statsT.tile([128, tb_max], F32, tag="statsT", name="stt")
                    nc.vector.tensor_copy(out=st_t[0:sc, 0:tbn], in_=pt[0:sc, 0:tbn])
                    if name == "p":
                        ppT[chi] = st_t
                    else:
                        qqT[chi] = st_t
            st["ppT"], st["qqT"] = ppT, qqT

        def emit_B_tile(st):
            j = st["jB"]
            if j >= st["tbn"]:
                return
            st["jB"] = j + 1
            tbn, bb, n0 = st["tbn"], st["b"], st["n0"]
            x_tiles, y_tiles = st["x"], st["y"]
            ppT, qqT = st["ppT"], st["qqT"]
            ot = st["ot"]
            nblk = j // nb
            if j % nb == 0:
                for chi in range(len(CH)):
                    ot[chi] = ostage.tile([128, nb, C], F32, tag="ostage", name=f"ot{chi}")
                    nc.vector.memset(ot[chi][0:1, 0:1, 0:1], 0.0)
            for chi, (s0, sc) in enumerate(CH):
                bank = ptok.tile([128, C], F32, tag="ptok", name="bank")
                nc.tensor.matmul(
                    out=bank[0:sc, :],
                    lhsT=y_tiles[j][:, s0 : s0 + sc].bitcast(F32),
                    rhs=ident[0:128, 0:128],
                    is_transpose=True,
                    start=True,
                    stop=fast_path,
                )
                if not fast_path:
                    nc.tensor.matmul(
                        out=bank[0:sc, :],
                        lhsT=st["qq"][j : j + 1, s0 : s0 + sc],
                        rhs=sc_row_sb[:, :],
                        start=False,
                        stop=True,
                    )
                xp = xtmp.tile([128, C], F32, tag="xtmp", name="xp")
                if fast_path:
                    nc.vector.tensor_scalar_add(
                        out=xp[0:sc, :],
                        in0=x_tiles[(nblk, chi)][0:sc, j % nb, :],
                        scalar1=qqT[chi][0:sc, j : j + 1],
                    )
                else:
                    nc.vector.tensor_tensor(
                        out=xp[0:sc, :],
                        in0=x_tiles[(nblk, chi)][0:sc, j % nb, :],
                        in1=gb_bc[0:sc, :],
                        op=OP.add,
                    )
                nc.vector.scalar_tensor_tensor(
                    out=ot[chi][0:sc, j % nb, :],
                    in0=bank[0:sc, :],
                    scalar=ppT[chi][0:sc, j : j + 1],
                    in1=xp[0:sc, :],
                    op0=OP.mult,
                    op1=OP.add,
                )
            if (j % nb == nb - 1) or (j == tbn - 1):
                nbw = (j % nb) + 1
                nst = n0 + nblk * nb
                for chi, (s0, sc) in enumerate(CH):
                    nc.sync.dma_start(
                        out=out_d[bb, s0 : s0 + sc, nst : nst + nbw, :],
                        in_=ot[chi][0:sc, 0:nbw, :],
                    )

        def drain_B(st):
            if st is None:
                return
            while st["jB"] < st["tbn"]:
                emit_B_tile(st)

        pending = None

        for b in range(b_per_core):
            cyc_sb = cycp.tile([C, s_total], F32, tag="cycp")
            nc.sync.dma_start(out=cyc_sb[:, :], in_=cyct_d[b, :, :])
            cyc_touch = cycp.tile([128, 1], F32, tag="cyct_touch")
            nc.vector.tensor_copy(out=cyc_touch[:, :], in_=cyc_sb[:, 0:1])

            for (n0, tbn) in n_batches():
                # ---------- PHASE A (with phase B of the previous batch interleaved) ----------
                s1_ps = pstat.tile([tb_max, s_total], F32, tag="pstat", name="s1_ps")
                s2_ps = pstat.tile([tb_max, s_total], F32, tag="pstat", name="s2_ps")
                st_cur = {"b": b, "n0": n0, "tbn": tbn, "x": {}, "y": {}, "jB": 0, "ot": {},
                          "s1": s1_ps, "s2": s2_ps}
                x_tiles, y_tiles = st_cur["x"], st_cur["y"]
                pend_stats = []

                def flush_stats(upto, tbn=tbn, s1_ps=s1_ps, s2_ps=s2_ps, pend_stats=pend_stats):
                    while pend_stats and pend_stats[0][0] <= upto:
                        jj, yt_, y2_ = pend_stats.pop(0)
                        win = ones_pad[:, tb_max - jj : 2 * tb_max - jj]
                        nc.tensor.matmul(
                            out=s1_ps[:, :], lhsT=win, rhs=yt_[:, :],
                            start=(jj == 0), stop=(jj == tbn - 1),
                        )
                        nc.tensor.matmul(
                            out=s2_ps[:, :], lhsT=win, rhs=y2_[:, :],
                            start=(jj == 0), stop=(jj == tbn - 1),
                        )

                for j in range(tbn):
                    nblk = j // nb
                    if j % nb == 0:
                        nbw = min(nb, tbn - nblk * nb)
                        for chi, (s0, sc) in enumerate(CH):
                            xt = xin.tile([128, nb, C], F32, tag="xin")
                            nc.sync.dma_start(
                                out=xt[0:sc, 0:nbw, :],
                                in_=x_d[b, s0 : s0 + sc, n0 + nblk * nb : n0 + nblk * nb + nbw, :],
                            )
                            x_tiles[(nblk, chi)] = xt

                    xT = pxT.tile([C, s_total], F32, tag="pxT", name="xT")
                    for chi, (s0, sc) in enumerate(CH):
                        nc.tensor.matmul(
                            out=xT[:, s0 : s0 + sc],
                            lhsT=x_tiles[(nblk, chi)][0:sc, j % nb, :],
                            rhs=ident[0:sc, 0:sc],
                            is_transpose=True,
                            start=True,
                            stop=True,
                        )
                    cb = comb.tile([C, SP], RDT, tag="comb", name="cb")
                    nc.vector.tensor_tensor(
                        out=cb[:, 1 : 1 + s_total], in0=xT[:, :], in1=cyc_sb[:, :], op=OP.mult
                    )
                    nc.vector.tensor_copy(out=cb[:, 0:1], in_=u2_col[:, 0:1])
                    nc.vector.tensor_copy(out=cb[:, SP - 1 : SP], in_=u2_col[:, 1:2])

                    z = pz.tile([C, s_total], F32, tag="pz", name="z")
                    if "noconv" in ablate:
                        nc.tensor.matmul(out=z[:, :], lhsT=a3_sb[:, 1, :],
                                         rhs=cb[:, 1 : 1 + s_total], start=True, stop=True)
                    else:
                        nc.tensor.matmul(out=z[:, :], lhsT=a3_sb[:, 1, :],
                                         rhs=cb[:, 1 : 1 + s_total], start=True, stop=False)
                        nc.tensor.matmul(out=z[:, :], lhsT=a3_sb[:, 0, :],
                                         rhs=cb[:, 0:s_total], start=False, stop=False)
                        nc.tensor.matmul(out=z[:, :], lhsT=a3_sb[:, 2, :],
                                         rhs=cb[:, 2 : 2 + s_total], start=False, stop=True)

                    yt = ypool.tile([C, s_total], RDT, tag="ypool", name="yt")
                    nc.scalar.activation(
                        out=yt[:, :], in_=z[:, :], func=act_fn, bias=gbias_col[:, :], scale=1.0
                    )
                    nc.scalar.activation(
                        out=yt[:, 0:1], in_=z[:, 0:1], func=act_fn, bias=gb_e0_col[:, :], scale=1.0
                    )
                    nc.scalar.activation(
                        out=yt[:, s_total - 1 : s_total], in_=z[:, s_total - 1 : s_total],
                        func=act_fn, bias=gb_e2_col[:, :], scale=1.0
                    )
                    y_tiles[j] = yt
                    if "nostats" not in ablate:
                        y2 = y2pool.tile([C, s_total], RDT, tag="y2pool", name="y2")
                        nc.scalar.activation(out=y2[:, :], in_=yt[:, :], func=AF.Square)
                        pend_stats.append((j, yt, y2))
                        flush_stats(j - 2)
                    if pipeline and pending is not None and (pipe_mode == 1 or j % 2 == 0):
                        emit_B_tile(pending)

                if "nostats" not in ablate:
                    flush_stats(tbn)

                if "nophb" in ablate or "nostats" in ablate:
                    for j0 in range(0, tbn, nb):
                        nbw = min(nb, tbn - j0)
                        for chi, (s0, sc) in enumerate(CH):
                            otx = ostage.tile([128, nb, C], F32, tag="ostage", name="otx")
                            nc.vector.tensor_copy(
                                out=otx[0:sc, 0:nbw, :],
                                in_=x_tiles[(j0 // nb, chi)][0:sc, 0:nbw, :],
                            )
                            nc.sync.dma_start(
                                out=out_d[b, s0 : s0 + sc, n0 + j0 : n0 + j0 + nbw, :],
                                in_=otx[0:sc, 0:nbw, :],
                            )
                    continue

                drain_B(pending)
                emit_stats_math_and_pq(st_cur)
                if pipeline:
                    pending = st_cur
                else:
                    drain_B(st_cur)
                    pending = None

        drain_B(pending)
    nc.compile()
    return nc


# ------------------------- host side -------------------------

def _host_prep(inputs):
    seasonal = np.asarray(inputs["seasonal_component"], dtype=np.float32)
    cycle_index = np.asarray(inputs["cycle_index"])
    cycle_data = np.asarray(inputs["cycle_data"], dtype=np.float32)
    W_c = np.asarray(inputs["W_c"], dtype=np.float32)
    lin_b = np.asarray(inputs["lin_b"], dtype=np.float32)
    b_c = np.asarray(inputs["b_c"], dtype=np.float32)
    conv_w = np.asarray(inputs["conv_w"], dtype=np.float32)
    conv_b = np.asarray(inputs["conv_b"], dtype=np.float32)
    ln_w = np.asarray(inputs["ln_w"], dtype=np.float32)
    ln_b = np.asarray(inputs["ln_b"], dtype=np.float32)
    gamma = float(np.asarray(inputs["gamma"]))

    b_, s_, n_, c_ = seasonal.shape
    cl = cycle_data.shape[0]

    idx = (np.asarray(cycle_index)[:, None] % cl + np.arange(s_)[None, :]) % cl
    cyc = cycle_data[idx]  # [B,S,C]
    cycT = np.ascontiguousarray(cyc.transpose(0, 2, 1))  # [B,C,S]

    w3 = conv_w[:, 0, :]  # [C,3]
    lb = lin_b + b_c
    a3 = np.ascontiguousarray(W_c.T[:, None, :] * w3.T[None, :, :]).astype(np.float32)

    gbias = lb * (w3[:, 0] + w3[:, 1] + w3[:, 2]) + conv_b
    gb_e0 = gbias - lb * w3[:, 0]
    gb_e2 = gbias - lb * w3[:, 2]

    fast_path = bool(np.all(ln_w == ln_w[0]) and np.all(ln_b == 0.0))
    gamma_sc = gamma * float(ln_w[0])
    sc_row = (gamma * ln_w).astype(np.float32)
    gb_row = (gamma * ln_b).astype(np.float32)

    vecs = np.stack([gb_e0, gb_e2, gbias, sc_row, gb_row], axis=0).astype(np.float32)
    return seasonal, cycT, a3, vecs, fast_path, gamma_sc


def _make_onespad(tb_max=TB_MAX):
    op = np.zeros((C, 2 * tb_max), np.float32)
    op[:, tb_max] = 1.0
    return op


_prog_cache = {}


def kernel(**inputs) -> np.ndarray:
    seasonal, cycT, a3, vecs, fast_path, gamma_sc = _host_prep(inputs)
    b_, s_, n_, c_ = seasonal.shape
    assert c_ == C
    bpc = b_ // NCORES

    key = (bpc, n_, s_, fast_path, gamma_sc)
    if key not in _prog_cache:
        _prog_cache[key] = build_program(
            b_per_core=bpc, n_total=n_, s_total=s_,
            gamma_sc=gamma_sc, fast_path=fast_path,
        )
    nc = _prog_cache[key]

    in_maps = []
    for i in range(NCORES):
        in_maps.append(
            {
                "x": np.ascontiguousarray(seasonal[i * bpc : (i + 1) * bpc]),
                "cyct": np.ascontiguousarray(cycT[i * bpc : (i + 1) * bpc]),
                "a3": a3,
                "vecs": vecs,
                "onespad": _make_onespad(),
            }
        )
    res = run_bass_kernel_spmd(nc, in_maps, list(range(NCORES)))
    outs = [res.results[i]["out"] for i in range(NCORES)]
    return np.concatenate(outs, axis=0)


# revision 32
# speedup vs baseline: 1.9886x; 1.0594x over previous
"""Trainium2 Bass kernel for nn_EnhancedSeasonalModule.

Computation (reference):
  cyc[b,s,:]   = cycle_data[(cycle_index[b]+s) % CL]
  combined     = seasonal * cyc                              [B,S,N,C]
  transformed  = combined @ W_c^T + (lin_b + b_c)            (einsum bsnc,dc->bsnd)
  z            = depthwise_conv1d_k3_same(transformed, conv_w)  over s, per (b,n)
  y            = gelu_exact(z + conv_b)
  ln           = layernorm_C(y) * ln_w + ln_b
  out          = seasonal + gamma * ln

Strategy: data-parallel over batch (2 of 16 per core, 8 cores).
Per (b,n) tile [C=128 x S=288], channels on partitions:
  - PE transposes [s,c] DMA tiles into [c,s]; DVE evacuates PSUM fused with
    the cyc multiply into a padded buffer whose edge columns hold
    u = -W^-1(lin_b+b_c), turning the depthwise conv + bias edge effects
    into exactly 3 PSUM-accumulated matmuls with A_k = diag(conv_w[:,0,k])@W_c;
    ACT applies exact GELU with the interior linear bias folded in; LN channel
    sums come from shifted-ones matmuls accumulated into per-batch PSUM stats
    banks; the finish transposes y back to token layout and applies
    out = y^T * rstd[s] + (x + q[s]) with one fused scalar_tensor_tensor.

fp32r matmul ISA rules (probed on hw): moving size must be even, PSUM out
offset 8B-aligned; rhs offset unrestricted. Producers feeding fp32r matmuls
must write float32r-typed outputs (engine rounds on write); DMA'd operands
must be declared float32r end-to-end.

walrus allows 1 sync wait per engine instruction; Bacc.compile()'s
generate_event_semaphores splits them, but we also warm up each PE/ACT/DVE
constant right after its DMA so steady-state instructions carry few waits.
"""

import numpy as np
from contextlib import ExitStack

import concourse.bass as bass
import concourse.bacc as bacc_mod
import concourse.tile as tile
from concourse import mybir
from concourse.bass_utils import run_bass_kernel_spmd
from concourse.masks import make_identity

F32 = mybir.dt.float32
AF = mybir.ActivationFunctionType
OP = mybir.AluOpType

B, S, N, C, CL = 16, 288, 170, 128, 24
LN_EPS = 1e-5
NCORES = 8
TB_MAX = 32


def _chunks(s_total):
    out = []
    s0 = 0
    while s0 < s_total:
        sc = min(128, s_total - s0)
        out.append((s0, sc))
        s0 += sc
    return out


def build_program(
    b_per_core: int,
    n_total: int,
    s_total: int,
    gamma_sc: float,
    tb_max: int = TB_MAX,
    nb: int = 16,
    use_f32r: bool = True,
    fast_path: bool = True,
    act_fn=None,
    repeat: int = 1,
    ablate: str = "",
    pipe_mode: int = 2,
):
    nc = bacc_mod.Bacc("TRN2", target_bir_lowering=False)
    if act_fn is None:
        act_fn = AF.Gelu

    CH = _chunks(s_total)
    SP = s_total + 2  # padded combined width (u | combined | u)

    a3_dt = mybir.dt.float32r if use_f32r else F32
    RDT = a3_dt

    x_d = nc.declare_dram_parameter("x", [b_per_core, s_total, n_total, C], F32, isOutput=False)
    cyct_d = nc.declare_dram_parameter("cyct", [b_per_core, C, s_total], F32, isOutput=False)
    a3_d = nc.declare_dram_parameter("a3", [C, 3, C], a3_dt, isOutput=False)
    # rows: 0=e0, 1=e2 (edge bias corrections), 2=gbias, 3=sc_row, 4=gb_row
    vec_d = nc.declare_dram_parameter("vecs", [5, C], F32, isOutput=False)
    onespad_d = nc.declare_dram_parameter("onespad", [C, 2 * tb_max], a3_dt, isOutput=False)
    out_d = nc.declare_dram_parameter("out", [b_per_core, s_total, n_total, C], F32, isOutput=True)

    def n_batches():
        res = []
        n0 = 0
        while n0 < n_total:
            t = min(tb_max, n_total - n0)
            res.append((n0, t))
            n0 += t
        return res

    with tile.TileContext(nc) as tc, ExitStack() as ctx:
        singles = ctx.enter_context(tc.tile_pool(name="singles", bufs=1))
        xin = ctx.enter_context(tc.tile_pool(name="xin", bufs=10))
        ypool = ctx.enter_context(tc.tile_pool(name="ypool", bufs=tb_max + 6))
        y2pool = ctx.enter_context(tc.tile_pool(name="y2pool", bufs=4))
        comb = ctx.enter_context(tc.tile_pool(name="comb", bufs=3))
        cycp = ctx.enter_context(tc.tile_pool(name="cycp", bufs=2))
        ostage = ctx.enter_context(tc.tile_pool(name="ostage", bufs=2 * len(CH)))
        stats = ctx.enter_context(tc.tile_pool(name="stats", bufs=8))
        statsT = ctx.enter_context(tc.tile_pool(name="statsT", bufs=2 * len(CH) * 2 + 2))
        xtmp = ctx.enter_context(tc.tile_pool(name="xtmp", bufs=4))

        pxT = ctx.enter_context(tc.tile_pool(name="pxT", bufs=2, space="PSUM"))
        pz = ctx.enter_context(tc.tile_pool(name="pz", bufs=2, space="PSUM"))
        pstat = ctx.enter_context(tc.tile_pool(name="pstat", bufs=2, space="PSUM"))
        ptok = ctx.enter_context(tc.tile_pool(name="ptok", bufs=2, space="PSUM"))

        # --- constants ---
        ident = singles.tile([128, 128], F32)
        make_identity(nc, ident[:, :])

        a3_sb = singles.tile([C, 3, C], a3_dt)
        nc.sync.dma_start(out=a3_sb[:, :, :], in_=a3_d[:, :, :])

        vec_sb = singles.tile([5, C], F32)
        nc.sync.dma_start(out=vec_sb[:, :], in_=vec_d[:, :])
        # zero pair for the padded combined edge columns
        u2_col = singles.tile([C, 2], F32)
        nc.vector.memset(u2_col[:, :], 0.0)
        gb_e0_col = singles.tile([C, 1], F32)
        nc.sync.dma_start(out=gb_e0_col[:, :], in_=vec_d[0:1, :].rearrange("a c -> c a"))
        gb_e2_col = singles.tile([C, 1], F32)
        nc.sync.dma_start(out=gb_e2_col[:, :], in_=vec_d[1:2, :].rearrange("a c -> c a"))
        gbias_col = singles.tile([C, 1], F32)
        nc.sync.dma_start(out=gbias_col[:, :], in_=vec_d[2:3, :].rearrange("a c -> c a"))
        sc_row_sb = singles.tile([1, C], F32)
        nc.sync.dma_start(out=sc_row_sb[:, :], in_=vec_d[3:4, :])

        ones1 = singles.tile([1, 1], F32)
        nc.vector.memset(ones1[:, :], 1.0)
        ones_col = singles.tile([128, 1], F32)
        nc.vector.memset(ones_col[:, :], 1.0)
        eps_col = singles.tile([128, 1], F32)
        nc.vector.memset(eps_col[:, :], LN_EPS)

        ones_pad = singles.tile([C, 2 * tb_max], a3_dt)
        nc.sync.dma_start(out=ones_pad[:, :], in_=onespad_d[:, :])

        # --- engine warm-ups: touch DMA'd constants once, one new dep per op ---
        pwarm = ptok.tile([128, C], F32, tag="ptok", name="pwarm")
        nc.tensor.matmul(out=pwarm[:, 0:128], lhsT=ident[:, :], rhs=ident[:, :],
                         is_transpose=True, start=True, stop=True)
        opw = min(128, 2 * tb_max)
        nc.tensor.matmul(out=pwarm[0:opw, 0:128], lhsT=ones_pad[:, 0:opw].bitcast(F32),
                         rhs=ident[:, :], is_transpose=True, start=True, stop=True)
        nc.tensor.matmul(out=pwarm[:, 0:1], lhsT=a3_sb[:, 1, :].bitcast(F32),
                         rhs=ones_col[:, :], start=True, stop=True)
        nc.tensor.matmul(out=pwarm[:, 0:1], lhsT=sc_row_sb[:, :], rhs=ones1[:, :],
                         start=True, stop=True)
        wscr = singles.tile([128, 1], F32)
        nc.scalar.activation(out=wscr[:, :], in_=gbias_col[:, :], func=AF.Square)
        nc.scalar.activation(out=wscr[:, :], in_=gb_e0_col[:, :], func=AF.Square)
        nc.scalar.activation(out=wscr[:, :], in_=gb_e2_col[:, :], func=AF.Square)
        wscr2 = singles.tile([128, 2], F32)
        nc.vector.tensor_copy(out=wscr2[:, :], in_=u2_col[:, :])

        if not fast_path:
            gb_bc = singles.tile([128, C], F32)
            src = vec_d[4, :]
            bcast = bass.AP(tensor=src.tensor, offset=src.offset, ap=[[0, 128]] + list(src.ap))
            nc.gpsimd.dma_start(out=gb_bc[:, :], in_=bcast)

        rep_ctx = tc.For_i(0, repeat, 1) if repeat > 1 else None
        if rep_ctx is not None:
            ctx.enter_context(rep_ctx)

        pipeline = (pipe_mode > 0) and not ablate

        def emit_stats_math_and_pq(st):
            tbn = st["tbn"]
            s1_ps, s2_ps = st["s1"], st["s2"]
            mu = stats.tile([tb_max, s_total], F32, tag="stats", name="mu")
            nc.vector.tensor_scalar_mul(out=mu[0:tbn, :], in0=s1_ps[0:tbn, :], scalar1=1.0 / C)
            var = stats.tile([tb_max, s_total], F32, tag="stats", name="var")
            nc.vector.tensor_scalar_mul(out=var[0:tbn, :], in0=s2_ps[0:tbn, :], scalar1=1.0 / C)
            msq = stats.tile([tb_max, s_total], F32, tag="stats", name="msq")
            nc.vector.tensor_tensor(out=msq[0:tbn, :], in0=mu[0:tbn, :], in1=mu[0:tbn, :], op=OP.mult)
            nc.vector.tensor_tensor(out=var[0:tbn, :], in0=var[0:tbn, :], in1=msq[0:tbn, :], op=OP.subtract)
            nc.scalar.activation(
                out=var[0:tbn, :], in_=var[0:tbn, :], func=AF.Sqrt,
                bias=eps_col[0:tbn, :], scale=1.0,
            )
            rstd = stats.tile([tb_max, s_total], F32, tag="stats", name="rstd")
            nc.vector.reciprocal(out=rstd[0:tbn, :], in_=var[0:tbn, :])
            if fast_path:
                pp = stats.tile([tb_max, s_total], F32, tag="stats", name="pp")
                nc.vector.tensor_scalar_mul(out=pp[0:tbn, :], in0=rstd[0:tbn, :], scalar1=float(gamma_sc))
            else:
                pp = rstd
            qq = stats.tile([tb_max, s_total], F32, tag="stats", name="qq")
            nc.vector.scalar_tensor_tensor(
                out=qq[0:tbn, :], in0=mu[0:tbn, :], scalar=-1.0, in1=pp[0:tbn, :],
                op0=OP.mult, op1=OP.mult,
            )
            st["qq"] = qq
            ppT = {}
            qqT = {}
            for chi, (s0, sc) in enumerate(CH):
                for name, srcm in (("p", pp), ("q", qq)):
                    pt = ptok.tile([128, C], F32, tag="ptok", name="pt")
                    nc.tensor.matmul(
                        out=pt[0:sc, 0:tbn],
                        lhsT=srcm[0:tbn, s0 : s0 + sc],
                        rhs=ident[0:tbn, 0:tbn],
                        is_transpose=True,
                        start=True,
                        stop=True,
                    )
                    st_t = statsT.tile([128, tb_max], F32, tag="statsT", name="stt")
                    nc.vector.tensor_copy(out=st_t[0:sc, 0:tbn], in_=pt[0:sc, 0:tbn])
                    if name == "p":
                        ppT[chi] = st_t
                    else:
                        qqT[chi] = st_t
            st["ppT"], st["qqT"] = ppT, qqT

        def emit_B_tile(st):
            j = st["jB"]
            if j >= st["tbn"]:
                return
            st["jB"] = j + 1
            tbn, bb, n0 = st["tbn"], st["b"], st["n0"]
            x_tiles, y_tiles = st["x"], st["y"]
            ppT, qqT = st["ppT"], st["qqT"]
            ot = st["ot"]
            nblk = j // nb
            if j % nb == 0:
                for chi in range(len(CH)):
                    ot[chi] = ostage.tile([128, nb, C], F32, tag="ostage", name=f"ot{chi}")
                    nc.vector.memset(ot[chi][0:1, 0:1, 0:1], 0.0)
            for chi, (s0, sc) in enumerate(CH):
                bank = ptok.tile([128, C], F32, tag="ptok", name="bank")
                nc.tensor.matmul(
                    out=bank[0:sc, :],
                    lhsT=y_tiles[j][:, s0 : s0 + sc].bitcast(F32),
                    rhs=ident[0:128, 0:128],
                    is_transpose=True,
                    start=True,
                    stop=fast_path,
                )
                if not fast_path:
                    nc.tensor.matmul(
                        out=bank[0:sc, :],
                        lhsT=st["qq"][j : j + 1, s0 : s0 + sc],
                        rhs=sc_row_sb[:, :],
                        start=False,
                        stop=True,
                    )
                xp = xtmp.tile([128, C], F32, tag="xtmp", name="xp")
                if fast_path:
                    nc.vector.tensor_scalar_add(
                        out=xp[0:sc, :],
                        in0=x_tiles[(nblk, chi)][0:sc, j % nb, :],
                        scalar1=qqT[chi][0:sc, j : j + 1],
                    )
                else:
                    nc.vector.tensor_tensor(
                        out=xp[0:sc, :],
                        in0=x_tiles[(nblk, chi)][0:sc, j % nb, :],
                        in1=gb_bc[0:sc, :],
                        op=OP.add,
                    )
                nc.vector.scalar_tensor_tensor(
                    out=ot[chi][0:sc, j % nb, :],
                    in0=bank[0:sc, :],
                    scalar=ppT[chi][0:sc, j : j + 1],
                    in1=xp[0:sc, :],
                    op0=OP.mult,
                    op1=OP.add,
                )
            if (j % nb == nb - 1) or (j == tbn - 1):
                nbw = (j % nb) + 1
                nst = n0 + nblk * nb
                for chi, (s0, sc) in enumerate(CH):
                    nc.sync.dma_start(
                        out=out_d[bb, s0 : s0 + sc, nst : nst + nbw, :],
                        in_=ot[chi][0:sc, 0:nbw, :],
                    )

        def drain_B(st):
            if st is None:
                return
            while st["jB"] < st["tbn"]:
                emit_B_tile(st)

        pending = None

        for b in range(b_per_core):
            cyc_sb = cycp.tile([C, s_total], F32, tag="cycp")
            nc.sync.dma_start(out=cyc_sb[:, :], in_=cyct_d[b, :, :])
            cyc_touch = cycp.tile([128, 1], F32, tag="cyct_touch")
            nc.vector.tensor_copy(out=cyc_touch[:, :], in_=cyc_sb[:, 0:1])

            for (n0, tbn) in n_batches():
                # ---------- PHASE A (with phase B of the previous batch interleaved) ----------
                s1_ps = pstat.tile([tb_max, s_total], F32, tag="pstat", name="s1_ps")
                s2_ps = pstat.tile([tb_max, s_total], F32, tag="pstat", name="s2_ps")
                st_cur = {"b": b, "n0": n0, "tbn": tbn, "x": {}, "y": {}, "jB": 0, "ot": {},
                          "s1": s1_ps, "s2": s2_ps}
                x_tiles, y_tiles = st_cur["x"], st_cur["y"]
                pend_stats = []

                def flush_stats(upto, tbn=tbn, s1_ps=s1_ps, s2_ps=s2_ps, pend_stats=pend_stats):
                    while pend_stats and pend_stats[0][0] <= upto:
                        jj, yt_, y2_ = pend_stats.pop(0)
                        win = ones_pad[:, tb_max - jj : 2 * tb_max - jj]
                        nc.tensor.matmul(
                            out=s1_ps[:, :], lhsT=win, rhs=yt_[:, :],
                            start=(jj == 0), stop=(jj == tbn - 1),
                        )
                        nc.tensor.matmul(
                            out=s2_ps[:, :], lhsT=win, rhs=y2_[:, :],
                            start=(jj == 0), stop=(jj == tbn - 1),
                        )

                for j in range(tbn):
                    nblk = j // nb
                    if j % nb == 0:
                        nbw = min(nb, tbn - nblk * nb)
                        for chi, (s0, sc) in enumerate(CH):
                            xt = xin.tile([128, nb, C], F32, tag="xin")
                            nc.sync.dma_start(
                                out=xt[0:sc, 0:nbw, :],
                                in_=x_d[b, s0 : s0 + sc, n0 + nblk * nb : n0 + nblk * nb + nbw, :],
                            )
                            x_tiles[(nblk, chi)] = xt

                    xT = pxT.tile([C, s_total], F32, tag="pxT", name="xT")
                    for chi, (s0, sc) in enumerate(CH):
                        nc.tensor.matmul(
                            out=xT[:, s0 : s0 + sc],
                            lhsT=x_tiles[(nblk, chi)][0:sc, j % nb, :],
                            rhs=ident[0:sc, 0:sc],
                            is_transpose=True,
                            start=True,
                            stop=True,
                        )
                    cb = comb.tile([C, SP], RDT, tag="comb", name="cb")
                    nc.vector.tensor_tensor(
                        out=cb[:, 1 : 1 + s_total], in0=xT[:, :], in1=cyc_sb[:, :], op=OP.mult
                    )
                    nc.vector.tensor_copy(out=cb[:, 0:1], in_=u2_col[:, 0:1])
                    nc.vector.tensor_copy(out=cb[:, SP - 1 : SP], in_=u2_col[:, 1:2])

                    z = pz.tile([C, s_total], F32, tag="pz", name="z")
                    if "noconv" in ablate:
                        nc.tensor.matmul(out=z[:, :], lhsT=a3_sb[:, 1, :],
                                         rhs=cb[:, 1 : 1 + s_total], start=True, stop=True)
                    else:
                        nc.tensor.matmul(out=z[:, :], lhsT=a3_sb[:, 1, :],
                                         rhs=cb[:, 1 : 1 + s_total], start=True, stop=False)
                        nc.tensor.matmul(out=z[:, :], lhsT=a3_sb[:, 0, :],
                                         rhs=cb[:, 0:s_total], start=False, stop=False)
                        nc.tensor.matmul(out=z[:, :], lhsT=a3_sb[:, 2, :],
                                         rhs=cb[:, 2 : 2 + s_total], start=False, stop=True)

                    yt = ypool.tile([C, s_total], RDT, tag="ypool", name="yt")
                    nc.scalar.activation(
                        out=yt[:, :], in_=z[:, :], func=act_fn, bias=gbias_col[:, :], scale=1.0
                    )
                    nc.scalar.activation(
                        out=yt[:, 0:1], in_=z[:, 0:1], func=act_fn, bias=gb_e0_col[:, :], scale=1.0
                    )
                    nc.scalar.activation(
                        out=yt[:, s_total - 1 : s_total], in_=z[:, s_total - 1 : s_total],
                        func=act_fn, bias=gb_e2_col[:, :], scale=1.0
                    )
                    y_tiles[j] = yt
                    if "nostats" not in ablate:
                        y2 = y2pool.tile([C, s_total], RDT, tag="y2pool", name="y2")
                        nc.scalar.activation(out=y2[:, :], in_=yt[:, :], func=AF.Square)
                        pend_stats.append((j, yt, y2))
                        flush_stats(j - 2)
                    if pipeline and pending is not None and (pipe_mode == 1 or j % 2 == 0):
                        emit_B_tile(pending)

                if "nostats" not in ablate:
                    flush_stats(tbn)

                if "nophb" in ablate or "nostats" in ablate:
                    for j0 in range(0, tbn, nb):
                        nbw = min(nb, tbn - j0)
                        for chi, (s0, sc) in enumerate(CH):
                            otx = ostage.tile([128, nb, C], F32, tag="ostage", name="otx")
                            nc.vector.tensor_copy(
                                out=otx[0:sc, 0:nbw, :],
                                in_=x_tiles[(j0 // nb, chi)][0:sc, 0:nbw, :],
                            )
                            nc.sync.dma_start(
                                out=out_d[b, s0 : s0 + sc, n0 + j0 : n0 + j0 + nbw, :],
                                in_=otx[0:sc, 0:nbw, :],
                            )
                    continue

                drain_B(pending)
                emit_stats_math_and_pq(st_cur)
                if pipeline:
                    pending = st_cur
                else:
                    drain_B(st_cur)
                    pending = None

        drain_B(pending)
    nc.compile()
    return nc


# ------------------------- host side -------------------------

def _host_prep(inputs):
    seasonal = np.asarray(inputs["seasonal_component"], dtype=np.float32)
    cycle_index = np.asarray(inputs["cycle_index"])
    cycle_data = np.asarray(inputs["cycle_data"], dtype=np.float32)
    W_c = np.asarray(inputs["W_c"], dtype=np.float32)
    lin_b = np.asarray(inputs["lin_b"], dtype=np.float32)
    b_c = np.asarray(inputs["b_c"], dtype=np.float32)
    conv_w = np.asarray(inputs["conv_w"], dtype=np.float32)
    conv_b = np.asarray(inputs["conv_b"], dtype=np.float32)
    ln_w = np.asarray(inputs["ln_w"], dtype=np.float32)
    ln_b = np.asarray(inputs["ln_b"], dtype=np.float32)
    gamma = float(np.asarray(inputs["gamma"]))

    b_, s_, n_, c_ = seasonal.shape
    cl = cycle_data.shape[0]

    idx = (np.asarray(cycle_index)[:, None] % cl + np.arange(s_)[None, :]) % cl
    cyc = cycle_data[idx]  # [B,S,C]
    cycT = np.ascontiguousarray(cyc.transpose(0, 2, 1))  # [B,C,S]

    w3 = conv_w[:, 0, :]  # [C,3]
    lb = lin_b + b_c
    a3 = np.ascontiguousarray(W_c.T[:, None, :] * w3.T[None, :, :]).astype(np.float32)

    gbias = lb * (w3[:, 0] + w3[:, 1] + w3[:, 2]) + conv_b
    gb_e0 = gbias - lb * w3[:, 0]
    gb_e2 = gbias - lb * w3[:, 2]

    fast_path = bool(np.all(ln_w == ln_w[0]) and np.all(ln_b == 0.0))
    gamma_sc = gamma * float(ln_w[0])
    sc_row = (gamma * ln_w).astype(np.float32)
    gb_row = (gamma * ln_b).astype(np.float32)

    vecs = np.stack([gb_e0, gb_e2, gbias, sc_row, gb_row], axis=0).astype(np.float32)
    return seasonal, cycT, a3, vecs, fast_path, gamma_sc


def _make_onespad(tb_max=TB_MAX):
    op = np.zeros((C, 2 * tb_max), np.float32)
    op[:, tb_max] = 1.0
    return op


_prog_cache = {}


def kernel(**inputs) -> np.ndarray:
    seasonal, cycT, a3, vecs, fast_path, gamma_sc = _host_prep(inputs)
    b_, s_, n_, c_ = seasonal.shape
    assert c_ == C
    bpc = b_ // NCORES

    key = (bpc, n_, s_, fast_path, gamma_sc)
    if key not in _prog_cache:
        _prog_cache[key] = build_program(
            b_per_core=bpc, n_total=n_, s_total=s_,
            gamma_sc=gamma_sc, fast_path=fast_path,
        )
    nc = _prog_cache[key]

    in_maps = []
    for i in range(NCORES):
        in_maps.append(
            {
                "x": np.ascontiguousarray(seasonal[i * bpc : (i + 1) * bpc]),
                "cyct": np.ascontiguousarray(cycT[i * bpc : (i + 1) * bpc]),
                "a3": a3,
                "vecs": vecs,
                "onespad": _make_onespad(),
            }
        )
    res = run_bass_kernel_spmd(nc, in_maps, list(range(NCORES)))
    outs = [res.results[i]["out"] for i in range(NCORES)]
    return np.concatenate(outs, axis=0)
